# revision 62
# baseline (speedup 1.0000x reference)
"""Trainium2 Bass kernel for nn_Couple_loss_62380105007762.

Loss = w0 * MSE + w1 * KLD + w2 * CE where
  sig(x)  = 2 * x[:, 0].sum(axis=F)                      (inverse SSQ-STFT, real channel only)
  MSE     = sum((sig(output_rec) - sig(target_rec))**2)
  KLD     = -0.5 * sum(1 + log_var - mean**2 - exp(log_var))
  CE      = mean cross-entropy(output_clas, target_clas)

Sharding: data-parallel over the batch dim (64 rows -> 8 cores x 8 rows).
Each core computes a weighted partial loss scalar; host sums the 8 partials
(the "psum" of per-shard losses).

Device strategy per core (memory-bound problem; only the REAL channel of the
rec tensors is ever read -> 2 x 8 MB of f32 traffic per core):
  - For each of the 8 batch rows: cast-DMA the [F=128, T=2048] o and t
    planes f32 -> bf16 into SBUF (gpsimd SWDGE, the only queue that can
    cast). HBM traffic is unchanged (16 MB reads) but SBUF write traffic
    halves, and every downstream engine touches half the bytes: the chip's
    power governor (throttle_activity_1, 50% util cap) was eating ~25% of
    DMA bandwidth when full-f32 compute overlapped the transfer window.
  - d = o - t on the DVE in bf16 (2x 16-bit rate, ~1us/row); values are
    N(0,1)-scale so bf16's ~0.4% element error averages to ~1e-3 on the
    final MSE, far inside the 2e-2 gate.
  - Partition-dim reduction over F via TensorE ones-matmul: psum[1, T] =
    (+1s)^T @ d in chunks of 512 (bf16 1 cycle/col).
  - Square-and-accumulate psum[1, T] -> scalar on ACT (f32 psum); PSUM
    ping-pong (bufs=2 x 4 banks) lets row b+1's matmuls run while row b's
    square drains. Row 7 is chunk-pipelined (2 x 1024 cols) so the work
    hanging off the last SWDGE transfer is a short hop.
  - KLD/CE terms (f32) on the tiny [8, 256]/[8, 5] shards with fused
    activation-accumulate ops; their DMAs ride the SP HWDGE queue (the
    scalar engine finishes last, and its end-of-kernel queue drain cost
    scales with entries processed).
  - Per-row/per-chunk partials (9 square cells + kc[8, 2]) are DMA'd out
    and combined on the host together with the other 7 cores' partials
    (the "psum" of per-shard losses) under the loss weights.
"""

import numpy as np
from contextlib import ExitStack

import concourse.bass as bass
import concourse.tile as tile
from concourse import mybir
from concourse.bass_utils import run_bass_kernel_spmd

N_CORES = 8
B, Z, F, T, C = 64, 256, 128, 2048, 5
BS = B // N_CORES  # batch rows per core
N_CHUNK = 512      # matmul moving-operand max free dim (fp32)

FP32 = mybir.dt.float32
FP32R = mybir.dt.float32r
BF16 = mybir.dt.bfloat16
AX = mybir.AxisListType
ALU = mybir.AluOpType
ACTF = mybir.ActivationFunctionType


def build_bass(legalize: bool = True):
    nc = bass.Bass()

    o_rec = nc.declare_dram_parameter("o_rec", [BS, F, T], FP32, isOutput=False)
    t_rec = nc.declare_dram_parameter("t_rec", [BS, F, T], FP32, isOutput=False)
    mean_in = nc.declare_dram_parameter("mean_in", [BS, Z], FP32, isOutput=False)
    logvar_in = nc.declare_dram_parameter("logvar_in", [BS, Z], FP32, isOutput=False)
    oclas = nc.declare_dram_parameter("oclas", [BS, C], FP32, isOutput=False)
    onehot = nc.declare_dram_parameter("onehot", [BS, C], FP32, isOutput=False)
    # +1 matmul weight column, bf16 to match the moving operand
    pm = nc.declare_dram_parameter("pm", [F, 1], BF16, isOutput=False)
    # partial results; the host does the final 8-core "psum" + weighting.
    # cells 0..6 = rows 0..6, cells 7..8 = row 7's two column chunks.
    out_sq = nc.declare_dram_parameter("out_sq", [1, BS + 1], FP32, isOutput=True)
    out_kc = nc.declare_dram_parameter("out_kc", [BS, 2], FP32, isOutput=True)

    with tile.TileContext(nc) as tc:
        with ExitStack() as ctx:
            const_pool = ctx.enter_context(tc.tile_pool(name="const", bufs=1))
            o_pool = ctx.enter_context(tc.tile_pool(name="opool", bufs=8))
            t_pool = ctx.enter_context(tc.tile_pool(name="tpool", bufs=8))
            # PSUM budget (8 banks): ps [1,T]=4 banks x bufs=2. The kc
            # partition-sum reuses the same ring after the loop drains.
            ps_pool = ctx.enter_context(tc.tile_pool(name="ps", bufs=2, space="PSUM"))
            junk_pool = ctx.enter_context(tc.tile_pool(name="junk", bufs=2))
            small = ctx.enter_context(tc.tile_pool(name="small", bufs=1))

            # small inputs ride the SP HWDGE queue: the scalar engine is the
            # last one busy at kernel end, and its final queue-drain cost
            # scales with how many DMAs its queue processed.
            pm_t = const_pool.tile([F, 1], BF16, tag="pm")
            nc.sync.dma_start(pm_t[:], pm[:, :])
            ones = pm_t[:, 0:1]

            # ---- small terms (KLD / CE) on their tiny shards ----
            m_t = small.tile([BS, Z], FP32, tag="m")
            lv_t = small.tile([BS, Z], FP32, tag="lv")
            oc_t = small.tile([BS, C], FP32, tag="oc")
            oh_t = small.tile([BS, C], FP32, tag="oh")
            nc.sync.dma_start(m_t[:], mean_in[:, :])
            nc.sync.dma_start(lv_t[:], logvar_in[:, :])
            nc.sync.dma_start(oc_t[:], oclas[:, :])
            nc.sync.dma_start(oh_t[:], onehot[:, :])

            # KLD rows: kld_row[b] = sum_z(log_var) - sum_z(mean^2) - sum_z(exp(log_var))
            msq_sum = small.tile([BS, 1], FP32, tag="msq")
            e_sum = small.tile([BS, 1], FP32, tag="esum")
            lv_sum = small.tile([BS, 1], FP32, tag="lvsum")
            kl_junk = small.tile([BS, Z], FP32, tag="klj")
            kl_junk2 = small.tile([BS, Z], FP32, tag="klj2")
            nc.vector.tensor_tensor(kl_junk[:], m_t[:], m_t[:], ALU.mult)
            nc.vector.reduce_sum(msq_sum[:], kl_junk[:], axis=AX.X)
            nc.scalar.activation(kl_junk2[:], lv_t[:], ACTF.Exp, accum_out=e_sum[:])
            nc.vector.reduce_sum(lv_sum[:], lv_t[:], axis=AX.X)

            # kc[:, 0] = kld_row, kc[:, 1] = ce_row
            kc = small.tile([BS, 2], FP32, tag="kc")
            kl_tmp = small.tile([BS, 1], FP32, tag="kltmp")
            nc.vector.tensor_tensor(kl_tmp[:], lv_sum[:], msq_sum[:], ALU.subtract)
            nc.vector.tensor_tensor(kc[:, 0:1], kl_tmp[:], e_sum[:], ALU.subtract)

            # CE rows: ce_row[b] = rowmax + log(sum(exp(oc - rowmax))) - oc[b, y_b]
            rowmax = small.tile([BS, 1], FP32, tag="rmax")
            nmax = small.tile([BS, 1], FP32, tag="nmax")
            sumexp = small.tile([BS, 1], FP32, tag="sexp")
            lse = small.tile([BS, 1], FP32, tag="lse")
            picked = small.tile([BS, 1], FP32, tag="picked")
            ce_junk = small.tile([BS, C], FP32, tag="cej")
            ce_junk2 = small.tile([BS, C], FP32, tag="cej2")
            ce_tmp = small.tile([BS, 1], FP32, tag="cetmp")
            nc.vector.reduce_max(rowmax[:], oc_t[:], axis=AX.X)
            nc.vector.tensor_scalar_mul(nmax[:], rowmax[:], -1.0)
            nc.scalar.activation(
                ce_junk[:], oc_t[:], ACTF.Exp, bias=nmax[:], accum_out=sumexp[:]
            )
            nc.scalar.activation(lse[:], sumexp[:], ACTF.Ln)
            nc.vector.tensor_tensor(ce_junk2[:], oc_t[:], oh_t[:], ALU.mult)
            nc.vector.reduce_sum(picked[:], ce_junk2[:], axis=AX.X)
            nc.vector.tensor_tensor(ce_tmp[:], rowmax[:], lse[:], ALU.add)
            nc.vector.tensor_tensor(kc[:, 1:2], ce_tmp[:], picked[:], ALU.subtract)

            # kc goes straight to the host (it sums the 8 rows); issue the
            # DMA as soon as kc is final — far off the critical tail.
            nc.sync.dma_start(out_kc[:, :], kc[:])

            # ---- main MSE stream ----
            # Rows 0-6 are whole [F, T] planes; row 7 is split into two
            # [F, 1024] o/t chunk pairs so the post-DMA tail chain is short.
            sq_acc = const_pool.tile([1, BS + 1], FP32, tag="sqacc")
            for b in range(BS - 2):
                o_tile = o_pool.tile([F, T], BF16, tag="o")
                t_tile = t_pool.tile([F, T], BF16, tag="t")
                # cast-DMA f32 -> bf16 (gpsimd SWDGE is the only caster)
                nc.gpsimd.dma_start(o_tile[:], o_rec[b, :, :])
                nc.gpsimd.dma_start(t_tile[:], t_rec[b, :, :])
                # d = o - t in-place on the o tile (DVE 2x 16-bit rate)
                nc.vector.tensor_tensor(
                    o_tile[:], o_tile[:], t_tile[:], ALU.subtract
                )
                ps = ps_pool.tile([1, T], FP32, tag="ps")
                for k in range(T // N_CHUNK):
                    sl = slice(k * N_CHUNK, (k + 1) * N_CHUNK)
                    nc.tensor.matmul(
                        ps[:, sl], ones, o_tile[:, sl], start=True, stop=True
                    )
                # square + accumulate sum over T on ACT (only one PSUM input
                # allowed per instruction, so DVE ps*ps is illegal)
                junk = junk_pool.tile([1, T], FP32, tag="junk")
                nc.scalar.activation(
                    junk[:], ps[:], ACTF.Square,
                    accum_out=sq_acc[0:1, b:b + 1],
                )

            # Row 6 rides the two otherwise-idle HWDGE queues as raw f32;
            # the bf16 downcast fuses into its DVE subtract (f32,f32 ->
            # bf16 out). Three queues with backlog per core also hedge
            # cross-core HBM arbitration during the contended window.
            b6 = BS - 2
            o6_f32 = o_pool.tile([F, T], FP32, tag="of32", bufs=1)
            t6_f32 = t_pool.tile([F, T], FP32, tag="tf32", bufs=1)
            nc.sync.dma_start(o6_f32[:], o_rec[b6, :, :])
            nc.scalar.dma_start(t6_f32[:], t_rec[b6, :, :])
            d6 = o_pool.tile([F, T], BF16, tag="o")
            nc.vector.tensor_tensor(d6[:], o6_f32[:], t6_f32[:], ALU.subtract)
            ps = ps_pool.tile([1, T], FP32, tag="ps")
            for k in range(T // N_CHUNK):
                sl = slice(k * N_CHUNK, (k + 1) * N_CHUNK)
                nc.tensor.matmul(
                    ps[:, sl], ones, d6[:, sl], start=True, stop=True
                )
            junk = junk_pool.tile([1, T], FP32, tag="junk")
            nc.scalar.activation(
                junk[:], ps[:], ACTF.Square,
                accum_out=sq_acc[0:1, b6:b6 + 1],
            )

            # [F, 1024] chunks keep 2 KB descriptor rows — big enough to
            # stay in the SWDGE line-rate regime (sub-2KB tail DMAs fell to
            # ~313 GB/s generation-paced trickle) while still shortening
            # the last subtract/matmul/square chain ~2x vs a full plane.
            b = BS - 1
            bounds = [0, 1024, 2048]
            for k in range(len(bounds) - 1):
                lo, hi = bounds[k], bounds[k + 1]
                w = hi - lo
                sl = slice(lo, hi)
                oc_k = o_pool.tile([F, w], BF16, tag="o")
                tc_k = t_pool.tile([F, w], BF16, tag="t")
                nc.gpsimd.dma_start(oc_k[:], o_rec[b, :, sl])
                nc.gpsimd.dma_start(tc_k[:], t_rec[b, :, sl])
                nc.vector.tensor_tensor(oc_k[:], oc_k[:], tc_k[:], ALU.subtract)
                ps = ps_pool.tile([1, w], FP32, tag="ps")
                for m0 in range(0, w, N_CHUNK):
                    m1 = min(m0 + N_CHUNK, w)
                    nc.tensor.matmul(
                        ps[:, m0:m1], ones, oc_k[:, m0:m1],
                        start=True, stop=True,
                    )
                junk = junk_pool.tile([1, w], FP32, tag="junk")
                nc.scalar.activation(
                    junk[:], ps[:], ACTF.Square,
                    accum_out=sq_acc[0:1, BS - 1 + k:BS + k],
                )

            # issue from the scalar engine: it runs the final read-
            # accumulator, so no cross-engine semaphore hop before the issue
            nc.scalar.dma_start(out_sq[:, :], sq_acc[:, :])

    if legalize:
        # CoreSim's race detector rejects the hoisted wait instructions
        # (no Tile fake sem updates), so sim runs build with legalize=False.
        _legalize_multi_waits(nc)
    # Populate .instr bytes for extended-ISA instructions
    # (tensor_tensor_reduce) — raw Bass skips Bacc's lowering pass and the
    # NEFF compiler fails with "ISA wrong length" without this.
    mybir.codegen_inst_isa_subclasses(nc)
    return nc


def _legalize_multi_waits(nc):
    """walrus rejects TPB compute instructions carrying more than one sync
    wait ("Too many sync wait commands" in the S3 encodings — hit for both
    Matmult/S3_LW and Activation/S3D3_AC). Hoist every wait of a multi-wait
    compute instruction onto standalone InstEventSemaphore instructions
    (exactly what `engine.wait_ge()` emits) inserted just before it on the
    same engine. DMA instructions keep their waits (DGE path handles many).
    """
    for fn in nc.m.functions:
        for blk in fn.blocks:
            new_insts = []
            for inst in blk.instructions:
                si = inst.sync_info
                tname = type(inst).__name__
                if (
                    si is not None
                    and si.on_wait
                    and len(si.on_wait) > 1
                    and tname != "InstEventSemaphore"
                ):
                    for i, w in enumerate(si.on_wait):
                        new_insts.append(
                            mybir.InstEventSemaphore(
                                name=f"{inst.name}_hoistw{i}",
                                engine=inst.engine,
                                ins=[],
                                outs=[],
                                sync_info=mybir.SyncInfo(on_wait=[w], on_update=[]),
                            )
                        )
                    inst.sync_info = mybir.SyncInfo(
                        on_wait=[], on_update=si.on_update
                    )
                new_insts.append(inst)
            blk.instructions = new_insts


_NC_CACHE = {}


def _get_nc():
    if "nc" not in _NC_CACHE:
        _NC_CACHE["nc"] = build_bass()
    return _NC_CACHE["nc"]


def make_in_maps(inputs) -> list[dict]:
    o = np.asarray(inputs["output_rec"], dtype=np.float32)
    t = np.asarray(inputs["target_rec"], dtype=np.float32)
    mean = np.asarray(inputs["mean"], dtype=np.float32)
    log_var = np.asarray(inputs["log_var"], dtype=np.float32)
    oclas = np.asarray(inputs["output_clas"], dtype=np.float32)
    tclas = np.asarray(inputs["target_clas"]).astype(np.int64)
    w = np.asarray(inputs["weight"], dtype=np.float32).astype(np.float64)

    # Only the real channel contributes to the inverse SSQ-STFT.
    o_real = np.ascontiguousarray(o[:, 0])  # [B, F, T]
    t_real = np.ascontiguousarray(t[:, 0])

    onehot = np.zeros((B, C), dtype=np.float32)
    onehot[np.arange(B), tclas] = 1.0

    from ml_dtypes import bfloat16 as _bf16
    pm = np.ones((F, 1), dtype=_bf16)

    in_maps = []
    for c in range(N_CORES):
        s = slice(c * BS, (c + 1) * BS)
        in_maps.append(
            {
                "o_rec": o_real[s],
                "t_rec": t_real[s],
                "mean_in": mean[s],
                "logvar_in": log_var[s],
                "oclas": oclas[s],
                "onehot": onehot[s],
                "pm": pm,
            }
        )
    return in_maps


def combine_partials(results, w) -> np.float32:
    """Host-side "psum": weighted sum of the 8 cores' partial outputs.

    Folds ISSQ_SCALE^2=4 (MSE), -0.5 and the sum-of-ones constant
    (KLD: 1 summed over B*Z elements), and 1/B (CE mean reduction).
    """
    w = np.asarray(w, dtype=np.float64)
    sq = sum(float(r["out_sq"].astype(np.float64).sum()) for r in results)
    kc = np.stack([r["out_kc"] for r in results])  # [cores, BS, 2]
    kld = float(kc[:, :, 0].sum())
    ce = float(kc[:, :, 1].sum())
    total = (4.0 * w[0] * sq
             - 0.5 * w[1] * (kld + B * Z)
             + w[2] * ce / B)
    return np.float32(total)


def kernel(**inputs) -> np.ndarray:
    in_maps = make_in_maps(inputs)
    nc = _get_nc()
    res = run_bass_kernel_spmd(nc, in_maps, list(range(N_CORES)))
    return combine_partials(res.results, inputs["weight"])


# revision 63
# speedup vs baseline: 1.1653x; 1.1653x over previous
"""Trainium2 Bass kernel for nn_Couple_loss_62380105007762.

Loss = w0 * MSE + w1 * KLD + w2 * CE where
  sig(x)  = 2 * x[:, 0].sum(axis=F)                      (inverse SSQ-STFT, real channel only)
  MSE     = sum((sig(output_rec) - sig(target_rec))**2)
  KLD     = -0.5 * sum(1 + log_var - mean**2 - exp(log_var))
  CE      = mean cross-entropy(output_clas, target_clas)

Sharding: data-parallel over the batch dim (64 rows -> 8 cores x 8 rows).
Each core computes a weighted partial loss scalar; host sums the 8 partials
(the "psum" of per-shard losses).

Device strategy per core (memory-bound problem; only the REAL channel of the
rec tensors is ever read -> 2 x 8 MB of f32 traffic per core):
  - For each of the 8 batch rows: cast-DMA the [F=128, T=2048] o and t
    planes f32 -> bf16 into SBUF (gpsimd SWDGE, the only queue that can
    cast). HBM traffic is unchanged (16 MB reads) but SBUF write traffic
    halves, and every downstream engine touches half the bytes: the chip's
    power governor (throttle_activity_1, 50% util cap) was eating ~25% of
    DMA bandwidth when full-f32 compute overlapped the transfer window.
  - d = o - t on the DVE in bf16 (2x 16-bit rate, ~1us/row); values are
    N(0,1)-scale so bf16's ~0.4% element error averages to ~1e-3 on the
    final MSE, far inside the 2e-2 gate.
  - Partition-dim reduction over F via TensorE ones-matmul: psum[1, T] =
    (+1s)^T @ d in chunks of 512 (bf16 1 cycle/col).
  - Square-and-accumulate psum[1, T] -> scalar on ACT (f32 psum); PSUM
    ping-pong (bufs=2 x 4 banks) lets row b+1's matmuls run while row b's
    square drains. Row 7 is chunk-pipelined (2 x 1024 cols) so the work
    hanging off the last SWDGE transfer is a short hop.
  - KLD/CE terms (f32) on the tiny [8, 256]/[8, 5] shards with fused
    activation-accumulate ops; their DMAs ride the SP HWDGE queue (the
    scalar engine finishes last, and its end-of-kernel queue drain cost
    scales with entries processed).
  - Per-row/per-chunk partials (9 square cells + kc[8, 2]) are DMA'd out
    and combined on the host together with the other 7 cores' partials
    (the "psum" of per-shard losses) under the loss weights.
"""

import numpy as np
from contextlib import ExitStack

import concourse.bass as bass
import concourse.tile as tile
from concourse import mybir
from concourse.bass_utils import run_bass_kernel_spmd

N_CORES = 8
B, Z, F, T, C = 64, 256, 128, 2048, 5
BS = B // N_CORES  # batch rows per core
N_CHUNK = 512      # matmul moving-operand max free dim (fp32)

FP32 = mybir.dt.float32
FP32R = mybir.dt.float32r
BF16 = mybir.dt.bfloat16
AX = mybir.AxisListType
ALU = mybir.AluOpType
ACTF = mybir.ActivationFunctionType


def build_bass(legalize: bool = True):
    nc = bass.Bass()

    o_rec = nc.declare_dram_parameter("o_rec", [BS, F, T], FP32, isOutput=False)
    t_rec = nc.declare_dram_parameter("t_rec", [BS, F, T], FP32, isOutput=False)
    mean_in = nc.declare_dram_parameter("mean_in", [BS, Z], FP32, isOutput=False)
    logvar_in = nc.declare_dram_parameter("logvar_in", [BS, Z], FP32, isOutput=False)
    oclas = nc.declare_dram_parameter("oclas", [BS, C], FP32, isOutput=False)
    onehot = nc.declare_dram_parameter("onehot", [BS, C], FP32, isOutput=False)
    # +1 matmul weight column, bf16 to match the moving operand
    pm = nc.declare_dram_parameter("pm", [F, 1], BF16, isOutput=False)
    # partial results; the host does the final 8-core "psum" + weighting.
    # cells 0..6 = rows 0..6, cells 7..8 = row 7's two column chunks.
    out_sq = nc.declare_dram_parameter("out_sq", [1, BS + 1], FP32, isOutput=True)
    out_kc = nc.declare_dram_parameter("out_kc", [BS, 2], FP32, isOutput=True)

    with tile.TileContext(nc) as tc:
        with ExitStack() as ctx:
            const_pool = ctx.enter_context(tc.tile_pool(name="const", bufs=1))
            o_pool = ctx.enter_context(tc.tile_pool(name="opool", bufs=8))
            t_pool = ctx.enter_context(tc.tile_pool(name="tpool", bufs=8))
            # PSUM budget (8 banks): ps [1,T]=4 banks x bufs=2. The kc
            # partition-sum reuses the same ring after the loop drains.
            ps_pool = ctx.enter_context(tc.tile_pool(name="ps", bufs=2, space="PSUM"))
            junk_pool = ctx.enter_context(tc.tile_pool(name="junk", bufs=2))
            small = ctx.enter_context(tc.tile_pool(name="small", bufs=1))

            # small inputs ride the SP HWDGE queue: the scalar engine is the
            # last one busy at kernel end, and its final queue-drain cost
            # scales with how many DMAs its queue processed.
            pm_t = const_pool.tile([F, 1], BF16, tag="pm")
            nc.sync.dma_start(pm_t[:], pm[:, :])
            ones = pm_t[:, 0:1]

            # ---- small terms (KLD / CE) on their tiny shards ----
            m_t = small.tile([BS, Z], FP32, tag="m")
            lv_t = small.tile([BS, Z], FP32, tag="lv")
            oc_t = small.tile([BS, C], FP32, tag="oc")
            oh_t = small.tile([BS, C], FP32, tag="oh")
            nc.sync.dma_start(m_t[:], mean_in[:, :])
            nc.sync.dma_start(lv_t[:], logvar_in[:, :])
            nc.sync.dma_start(oc_t[:], oclas[:, :])
            nc.sync.dma_start(oh_t[:], onehot[:, :])

            # KLD rows: kld_row[b] = sum_z(log_var) - sum_z(mean^2) - sum_z(exp(log_var))
            msq_sum = small.tile([BS, 1], FP32, tag="msq")
            e_sum = small.tile([BS, 1], FP32, tag="esum")
            lv_sum = small.tile([BS, 1], FP32, tag="lvsum")
            kl_junk = small.tile([BS, Z], FP32, tag="klj")
            kl_junk2 = small.tile([BS, Z], FP32, tag="klj2")
            nc.vector.tensor_tensor(kl_junk[:], m_t[:], m_t[:], ALU.mult)
            nc.vector.reduce_sum(msq_sum[:], kl_junk[:], axis=AX.X)
            nc.scalar.activation(kl_junk2[:], lv_t[:], ACTF.Exp, accum_out=e_sum[:])
            nc.vector.reduce_sum(lv_sum[:], lv_t[:], axis=AX.X)

            # kc[:, 0] = kld_row, kc[:, 1] = ce_row
            kc = small.tile([BS, 2], FP32, tag="kc")
            kl_tmp = small.tile([BS, 1], FP32, tag="kltmp")
            nc.vector.tensor_tensor(kl_tmp[:], lv_sum[:], msq_sum[:], ALU.subtract)
            nc.vector.tensor_tensor(kc[:, 0:1], kl_tmp[:], e_sum[:], ALU.subtract)

            # CE rows: ce_row[b] = rowmax + log(sum(exp(oc - rowmax))) - oc[b, y_b]
            rowmax = small.tile([BS, 1], FP32, tag="rmax")
            nmax = small.tile([BS, 1], FP32, tag="nmax")
            sumexp = small.tile([BS, 1], FP32, tag="sexp")
            lse = small.tile([BS, 1], FP32, tag="lse")
            picked = small.tile([BS, 1], FP32, tag="picked")
            ce_junk = small.tile([BS, C], FP32, tag="cej")
            ce_junk2 = small.tile([BS, C], FP32, tag="cej2")
            ce_tmp = small.tile([BS, 1], FP32, tag="cetmp")
            nc.vector.reduce_max(rowmax[:], oc_t[:], axis=AX.X)
            nc.vector.tensor_scalar_mul(nmax[:], rowmax[:], -1.0)
            nc.scalar.activation(
                ce_junk[:], oc_t[:], ACTF.Exp, bias=nmax[:], accum_out=sumexp[:]
            )
            nc.scalar.activation(lse[:], sumexp[:], ACTF.Ln)
            nc.vector.tensor_tensor(ce_junk2[:], oc_t[:], oh_t[:], ALU.mult)
            nc.vector.reduce_sum(picked[:], ce_junk2[:], axis=AX.X)
            nc.vector.tensor_tensor(ce_tmp[:], rowmax[:], lse[:], ALU.add)
            nc.vector.tensor_tensor(kc[:, 1:2], ce_tmp[:], picked[:], ALU.subtract)

            # kc goes straight to the host (it sums the 8 rows); issue the
            # DMA as soon as kc is final — far off the critical tail.
            nc.sync.dma_start(out_kc[:, :], kc[:])

            # ---- main MSE stream ----
            # Rows 0-6 are whole [F, T] planes; row 7 is split into two
            # [F, 1024] o/t chunk pairs so the post-DMA tail chain is short.
            sq_acc = const_pool.tile([1, BS + 1], FP32, tag="sqacc")
            for b in range(BS - 1):
                o_tile = o_pool.tile([F, T], BF16, tag="o")
                t_tile = t_pool.tile([F, T], BF16, tag="t")
                # cast-DMA f32 -> bf16 (gpsimd SWDGE is the only caster)
                nc.gpsimd.dma_start(o_tile[:], o_rec[b, :, :])
                nc.gpsimd.dma_start(t_tile[:], t_rec[b, :, :])
                # d = o - t in-place on the o tile (DVE 2x 16-bit rate)
                nc.vector.tensor_tensor(
                    o_tile[:], o_tile[:], t_tile[:], ALU.subtract
                )
                ps = ps_pool.tile([1, T], FP32, tag="ps")
                for k in range(T // N_CHUNK):
                    sl = slice(k * N_CHUNK, (k + 1) * N_CHUNK)
                    nc.tensor.matmul(
                        ps[:, sl], ones, o_tile[:, sl], start=True, stop=True
                    )
                # square + accumulate sum over T on ACT (only one PSUM input
                # allowed per instruction, so DVE ps*ps is illegal)
                junk = junk_pool.tile([1, T], FP32, tag="junk")
                nc.scalar.activation(
                    junk[:], ps[:], ACTF.Square,
                    accum_out=sq_acc[0:1, b:b + 1],
                )

            # [F, 1024] chunks keep 2 KB descriptor rows — big enough to
            # stay in the SWDGE line-rate regime (sub-2KB tail DMAs fell to
            # ~313 GB/s generation-paced trickle) while still shortening
            # the last subtract/matmul/square chain ~2x vs a full plane.
            b = BS - 1
            bounds = [0, 1024, 2048]
            for k in range(len(bounds) - 1):
                lo, hi = bounds[k], bounds[k + 1]
                w = hi - lo
                sl = slice(lo, hi)
                oc_k = o_pool.tile([F, w], BF16, tag="o")
                tc_k = t_pool.tile([F, w], BF16, tag="t")
                nc.gpsimd.dma_start(oc_k[:], o_rec[b, :, sl])
                nc.gpsimd.dma_start(tc_k[:], t_rec[b, :, sl])
                nc.vector.tensor_tensor(oc_k[:], oc_k[:], tc_k[:], ALU.subtract)
                ps = ps_pool.tile([1, w], FP32, tag="ps")
                for m0 in range(0, w, N_CHUNK):
                    m1 = min(m0 + N_CHUNK, w)
                    nc.tensor.matmul(
                        ps[:, m0:m1], ones, oc_k[:, m0:m1],
                        start=True, stop=True,
                    )
                junk = junk_pool.tile([1, w], FP32, tag="junk")
                nc.scalar.activation(
                    junk[:], ps[:], ACTF.Square,
                    accum_out=sq_acc[0:1, BS - 1 + k:BS + k],
                )

            # issue from the scalar engine: it runs the final read-
            # accumulator, so no cross-engine semaphore hop before the issue
            nc.scalar.dma_start(out_sq[:, :], sq_acc[:, :])

    if legalize:
        # CoreSim's race detector rejects the hoisted wait instructions
        # (no Tile fake sem updates), so sim runs build with legalize=False.
        _legalize_multi_waits(nc)
    # Populate .instr bytes for extended-ISA instructions
    # (tensor_tensor_reduce) — raw Bass skips Bacc's lowering pass and the
    # NEFF compiler fails with "ISA wrong length" without this.
    mybir.codegen_inst_isa_subclasses(nc)
    return nc


def _legalize_multi_waits(nc):
    """walrus rejects TPB compute instructions carrying more than one sync
    wait ("Too many sync wait commands" in the S3 encodings — hit for both
    Matmult/S3_LW and Activation/S3D3_AC). Hoist every wait of a multi-wait
    compute instruction onto standalone InstEventSemaphore instructions
    (exactly what `engine.wait_ge()` emits) inserted just before it on the
    same engine. DMA instructions keep their waits (DGE path handles many).
    """
    for fn in nc.m.functions:
        for blk in fn.blocks:
            new_insts = []
            for inst in blk.instructions:
                si = inst.sync_info
                tname = type(inst).__name__
                if (
                    si is not None
                    and si.on_wait
                    and len(si.on_wait) > 1
                    and tname != "InstEventSemaphore"
                ):
                    for i, w in enumerate(si.on_wait):
                        new_insts.append(
                            mybir.InstEventSemaphore(
                                name=f"{inst.name}_hoistw{i}",
                                engine=inst.engine,
                                ins=[],
                                outs=[],
                                sync_info=mybir.SyncInfo(on_wait=[w], on_update=[]),
                            )
                        )
                    inst.sync_info = mybir.SyncInfo(
                        on_wait=[], on_update=si.on_update
                    )
                new_insts.append(inst)
            blk.instructions = new_insts


_NC_CACHE = {}


def _get_nc():
    if "nc" not in _NC_CACHE:
        _NC_CACHE["nc"] = build_bass()
    return _NC_CACHE["nc"]


def make_in_maps(inputs) -> list[dict]:
    o = np.asarray(inputs["output_rec"], dtype=np.float32)
    t = np.asarray(inputs["target_rec"], dtype=np.float32)
    mean = np.asarray(inputs["mean"], dtype=np.float32)
    log_var = np.asarray(inputs["log_var"], dtype=np.float32)
    oclas = np.asarray(inputs["output_clas"], dtype=np.float32)
    tclas = np.asarray(inputs["target_clas"]).astype(np.int64)
    w = np.asarray(inputs["weight"], dtype=np.float32).astype(np.float64)

    # Only the real channel contributes to the inverse SSQ-STFT.
    o_real = np.ascontiguousarray(o[:, 0])  # [B, F, T]
    t_real = np.ascontiguousarray(t[:, 0])

    onehot = np.zeros((B, C), dtype=np.float32)
    onehot[np.arange(B), tclas] = 1.0

    from ml_dtypes import bfloat16 as _bf16
    pm = np.ones((F, 1), dtype=_bf16)

    in_maps = []
    for c in range(N_CORES):
        s = slice(c * BS, (c + 1) * BS)
        in_maps.append(
            {
                "o_rec": o_real[s],
                "t_rec": t_real[s],
                "mean_in": mean[s],
                "logvar_in": log_var[s],
                "oclas": oclas[s],
                "onehot": onehot[s],
                "pm": pm,
            }
        )
    return in_maps


def combine_partials(results, w) -> np.float32:
    """Host-side "psum": weighted sum of the 8 cores' partial outputs.

    Folds ISSQ_SCALE^2=4 (MSE), -0.5 and the sum-of-ones constant
    (KLD: 1 summed over B*Z elements), and 1/B (CE mean reduction).
    """
    w = np.asarray(w, dtype=np.float64)
    sq = sum(float(r["out_sq"].astype(np.float64).sum()) for r in results)
    kc = np.stack([r["out_kc"] for r in results])  # [cores, BS, 2]
    kld = float(kc[:, :, 0].sum())
    ce = float(kc[:, :, 1].sum())
    total = (4.0 * w[0] * sq
             - 0.5 * w[1] * (kld + B * Z)
             + w[2] * ce / B)
    return np.float32(total)


def kernel(**inputs) -> np.ndarray:
    in_maps = make_in_maps(inputs)
    nc = _get_nc()
    res = run_bass_kernel_spmd(nc, in_maps, list(range(N_CORES)))
    return combine_partials(res.results, inputs["weight"])
